# revision 23
# baseline (speedup 1.0000x reference)
"""Per-row bincount (BagOfWords) Trainium2 kernel.

Full input: inputs [16384, 512] int32, token ids in [0, 1101).
Full output: [16384, 1100] fp32, counts[r, t-1] = #{s : inputs[r, s] == t}.

Sharding: pure data parallel over the batch axis across 8 NeuronCores
(2048 rows per core).

Per-core algorithm (histogram as factorized outer-product-accumulate):
  t = 35*a + b with a in [0,32), b in [0,35)  (32*35 = 1120 >= 1101)
  counts[r, 35a+b] = sum_s onehot32(a_rs)[a] * onehot35(b_rs)[b]
which is a matmul over the token axis s. Rows are packed 4-per-matmul
block-diagonally: stationary = [128 s, 4 rows x 32 a-bins] one-hots
(built by GPSIMD local_scatter, contiguous so FWL kicks in), moving =
[128 s, 4 rows x 35 b-bins] one-hots (built by DVE tensor_tensor
is_equal against an iota tile), PSUM [128, 140] accumulates over the 4
s-chunks; the 4 diagonal [32, 35] blocks are each one row's histogram.
ScalarE copies PSUM->SBUF staging; strided HWDGE DMAs scatter the
diagonal blocks to the [2048, 1100] output (dropping bin t=0).
"""
import sys

sys.path.insert(0, "/opt/trn_rl_repo")

import numpy as np

import concourse.bass as bass
import concourse.tile as tile
from concourse import bacc, mybir
from concourse.bass_utils import run_bass_kernel_spmd

P = 128
S = 512          # tokens per row
B_CORE = 2048    # rows per core
N_CORES = 8
NB = 35          # b bins (t % 35)
NA = 32          # a bins (t // 35)
T_OUT = 1100
G = 32           # rows per one-hot generation group
RPB = 128        # rows per batch
N_BATCH = B_CORE // RPB

f32 = mybir.dt.float32
bf16 = mybir.dt.bfloat16
i16 = mybir.dt.int16
i32 = mybir.dt.int32
fp16 = mybir.dt.float16


def build_nc(n_batches=N_BATCH):
    nc = bacc.Bacc("TRN2", target_bir_lowering=False, debug=False,
                   num_devices=N_CORES)
    x = nc.dram_tensor("x", [B_CORE, S], i32, kind="ExternalInput")
    counts = nc.dram_tensor("counts", [B_CORE, T_OUT], fp16,
                            kind="ExternalOutput")
    with tile.TileContext(nc) as tc:
        build_body(nc, tc, x, counts, n_batches)
    nc.compile()
    return nc


def build_body(nc, tc, x, counts, n_batches):
    import contextlib
    ctx = contextlib.ExitStack()
    with ctx:
        const_pool = ctx.enter_context(tc.tile_pool(name="const", bufs=1))
        x_pool = ctx.enter_context(tc.tile_pool(name="x", bufs=6))
        deriv_pool = ctx.enter_context(tc.tile_pool(name="deriv", bufs=6))
        tr_pool = ctx.enter_context(tc.tile_pool(name="tr", bufs=12))
        oh_pool = ctx.enter_context(tc.tile_pool(name="oh", bufs=8))
        st_pool = ctx.enter_context(tc.tile_pool(name="st", bufs=3))
        psum_pool = ctx.enter_context(
            tc.tile_pool(name="psum", bufs=2, space="PSUM"))

        # --- constants ---
        # iota over b bins, b-outer/row-inner: value k at [p, k*G + r]
        iota_b_i = const_pool.tile([P, NB * G], i16)
        nc.gpsimd.iota(iota_b_i[:], pattern=[[1, NB], [0, G]],
                       channel_multiplier=0)
        iota_b = const_pool.tile([P, NB * G], bf16)
        nc.vector.tensor_copy(iota_b[:], iota_b_i[:])
        # row offsets for a-side scatter indices: 32*(r % 16) over 128 cols
        rowoff = const_pool.tile([P, RPB], i16)
        nc.gpsimd.iota(rowoff[:], pattern=[[0, RPB // G], [NA, G]],
                       channel_multiplier=0)
        ones_t = const_pool.tile([P, G], bf16)
        nc.vector.memset(ones_t[:], 1.0)

        counts_v = counts.rearrange("(n f) t -> n f t", f=4)  # [512, 4, 1100]

        for bi in range(n_batches):
            # --- load + derive a, b ---
            xt = x_pool.tile([P, S], i32)
            nc.sync.dma_start(out=xt[:], in_=x[bi * RPB:(bi + 1) * RPB, :])
            # a = x // 35 via magic-number division: (x * 937) >> 15,
            # exact for all x < 42477; b = x - 35 * a.
            xm = deriv_pool.tile([P, S], i32)
            nc.vector.tensor_scalar(xm[:], xt[:], 937, None,
                                    op0=mybir.AluOpType.mult)
            a_i = deriv_pool.tile([P, S], i32)
            nc.vector.tensor_scalar(a_i[:], xm[:], 15, None,
                                    op0=mybir.AluOpType.logical_shift_right)
            a_bf = deriv_pool.tile([P, S], bf16)
            nc.vector.tensor_copy(a_bf[:], a_i[:])
            b_bf = deriv_pool.tile([P, S], bf16)
            nc.vector.scalar_tensor_tensor(
                b_bf[:], a_i[:], -35.0, xt[:],
                op0=mybir.AluOpType.mult, op1=mybir.AluOpType.add)

            # --- transpose a, b to [s, row] ---
            aT = []
            bT = []
            for c in range(4):
                at = tr_pool.tile([P, RPB], bf16, tag="aT")
                nc.sync.dma_start(out=at[:],
                                  in_=a_bf[:, c * P:(c + 1) * P],
                                  transpose=True)
                aT.append(at)
                bt = tr_pool.tile([P, RPB], bf16, tag="bT")
                nc.sync.dma_start(out=bt[:],
                                  in_=b_bf[:, c * P:(c + 1) * P],
                                  transpose=True)
                bT.append(bt)

            # a-side scatter indices for all rows of each chunk: 32*(r%16)+a
            idx = []
            for c in range(4):
                ix = tr_pool.tile([P, RPB], i16, tag="idx")
                nc.vector.tensor_tensor(ix[:], aT[c][:], rowoff[:],
                                        op=mybir.AluOpType.add)
                idx.append(ix)

            st = st_pool.tile([P, 140 * 32], fp16)

            for g in range(RPB // G):
                # --- one-hots for this 16-row group, all 4 s-chunks ---
                oa = []
                ob = []
                for c in range(4):
                    o_a = oh_pool.tile([P, G * NA], bf16, tag="oa")
                    nc.gpsimd.local_scatter(
                        out_ap=o_a[:],
                        data_ap=ones_t[:],
                        idxs_ap=idx[c][:, g * G:(g + 1) * G],
                        channels=P, num_elems=G * NA, num_idxs=G)
                    oa.append(o_a)
                    o_b = oh_pool.tile([P, NB * G], bf16, tag="ob")
                    bsl = bT[c][:, g * G:(g + 1) * G]
                    nc.vector.tensor_tensor(
                        o_b[:], bsl[:, None, :].to_broadcast([P, NB, G]),
                        iota_b[:], op=mybir.AluOpType.is_equal)
                    ob.append(o_b)
                for wave in range(G // 16):
                    pss = []
                    for m in range(4):
                        ps_t = psum_pool.tile([P, 140], f32, space="PSUM",
                                              tag=f"ps{m}", name=f"ps{m}")
                        pss.append(ps_t)
                    for c in range(4):
                        for m in range(4):
                            mg = wave * 4 + m
                            # moving columns streamed in native (k, r) order:
                            # innermost dim contiguous in SBUF (fast fetch);
                            # the PSUM->SBUF copy undoes the permutation.
                            rhs = ob[c][:].rearrange(
                                "p (k r) -> p k r",
                                k=NB)[:, :, mg * 4:(mg + 1) * 4]
                            nc.tensor.matmul(
                                pss[m][:],
                                lhsT=oa[c][:, mg * P:(mg + 1) * P],
                                rhs=rhs,
                                start=(c == 0), stop=(c == 3))
                    for m in range(4):
                        grp = g * (G // 4) + wave * 4 + m
                        st_sl = st[:, 140 * grp:140 * (grp + 1)].rearrange(
                            "p (r k) -> p r k", r=4)
                        nc.scalar.copy(
                            st_sl,
                            pss[m][:].rearrange("p (k r) -> p r k", k=NB))

            # --- scatter diagonal blocks to HBM ---
            stv = st[:].rearrange("p (grp r k) -> p grp r k", grp=32, r=4)
            cb = counts_v[bi * 32:(bi + 1) * 32]  # [32, 4, 1100]
            for j in range(4):
                # a in [1, 31): 30 partitions x 35 cols -> t-1 in [34, 1084)
                dst = cb[:, j, 34:1084].rearrange("r (a b) -> a r b", a=30)
                nc.sync.dma_start(
                    out=dst, in_=stv[32 * j + 1:32 * j + 31, :, j, :])
                # a == 0: b in [1, 35) -> t-1 in [0, 34)
                nc.sync.dma_start(
                    out=cb[None, :, j, 0:34],
                    in_=stv[32 * j:32 * j + 1, :, j, 1:35])
                # a == 31: b in [0, 16) -> t-1 in [1084, 1100)
                nc.sync.dma_start(
                    out=cb[None, :, j, 1084:1100],
                    in_=stv[32 * j + 31:32 * j + 32, :, j, 0:16])


_NC_CACHE = {}


def _get_nc():
    if "nc" not in _NC_CACHE:
        _NC_CACHE["nc"] = build_nc()
    return _NC_CACHE["nc"]


def kernel(**inputs):
    x = np.asarray(inputs["inputs"])
    in_dtype = x.dtype
    x = np.ascontiguousarray(x.astype(np.int32))
    shards = x.reshape(N_CORES, B_CORE, S)
    nc = _get_nc()
    in_maps = [{"x": shards[i]} for i in range(N_CORES)]
    res = run_bass_kernel_spmd(nc, in_maps, core_ids=list(range(N_CORES)))
    out = np.concatenate([r["counts"] for r in res.results], axis=0)
    return out.astype(np.float32)


if __name__ == "__main__":
    rng = np.random.default_rng(0)
    x = rng.integers(0, 1101, size=(16384, 512), dtype=np.int32)
    out = kernel(inputs=x)
    # numpy reference
    exp = np.zeros((16384, 1101), np.float32)
    for r in range(0, 16384, 4096):
        blk = x[r:r + 4096]
        idx = np.arange(blk.shape[0])[:, None]
        np.add.at(exp[r:r + 4096], (idx, blk), 1.0)
    exp = exp[:, 1:]
    print("match:", np.array_equal(out, exp),
          "maxerr:", np.abs(out - exp).max())


# revision 24
# speedup vs baseline: 1.0486x; 1.0486x over previous
"""Per-row bincount (BagOfWords) Trainium2 kernel.

Full input: inputs [16384, 512] int32, token ids in [0, 1101).
Full output: [16384, 1100] fp32, counts[r, t-1] = #{s : inputs[r, s] == t}.

Sharding: pure data parallel over the batch axis across 8 NeuronCores
(2048 rows per core).

Per-core algorithm (histogram as factorized outer-product-accumulate):
  t = 35*a + b with a in [0,32), b in [0,35)  (32*35 = 1120 >= 1101)
  counts[r, 35a+b] = sum_s onehot32(a_rs)[a] * onehot35(b_rs)[b]
which is a matmul over the token axis s. Rows are packed 4-per-matmul
block-diagonally: stationary = [128 s, 4 rows x 32 a-bins] one-hots
(built by GPSIMD local_scatter, contiguous so FWL kicks in), moving =
[128 s, 4 rows x 35 b-bins] one-hots (built by DVE tensor_tensor
is_equal against an iota tile), PSUM [128, 140] accumulates over the 4
s-chunks; the 4 diagonal [32, 35] blocks are each one row's histogram.
ScalarE copies PSUM->SBUF staging; strided HWDGE DMAs scatter the
diagonal blocks to the [2048, 1100] output (dropping bin t=0).
"""
import sys

sys.path.insert(0, "/opt/trn_rl_repo")

import numpy as np

import concourse.bass as bass
import concourse.tile as tile
from concourse import bacc, mybir
from concourse.bass_utils import run_bass_kernel_spmd

P = 128
S = 512          # tokens per row
B_CORE = 2048    # rows per core
N_CORES = 8
NB = 35          # b bins (t % 35)
NA = 32          # a bins (t // 35)
T_OUT = 1100
G = 32           # rows per one-hot generation group
RPB = 128        # rows per batch
N_BATCH = B_CORE // RPB

f32 = mybir.dt.float32
bf16 = mybir.dt.bfloat16
i16 = mybir.dt.int16
i32 = mybir.dt.int32
fp16 = mybir.dt.float16


def build_nc(n_batches=N_BATCH):
    nc = bacc.Bacc("TRN2", target_bir_lowering=False, debug=False,
                   num_devices=N_CORES)
    x = nc.dram_tensor("x", [B_CORE, S], i32, kind="ExternalInput")
    counts = nc.dram_tensor("counts", [B_CORE, T_OUT], fp16,
                            kind="ExternalOutput")
    with tile.TileContext(nc) as tc:
        build_body(nc, tc, x, counts, n_batches)
    nc.compile()
    return nc


def build_body(nc, tc, x, counts, n_batches):
    import contextlib
    ctx = contextlib.ExitStack()
    with ctx:
        const_pool = ctx.enter_context(tc.tile_pool(name="const", bufs=1))
        x_pool = ctx.enter_context(tc.tile_pool(name="x", bufs=4))
        deriv_pool = ctx.enter_context(tc.tile_pool(name="deriv", bufs=4))
        tr_pool = ctx.enter_context(tc.tile_pool(name="tr", bufs=12))
        oh_pool = ctx.enter_context(tc.tile_pool(name="oh", bufs=8))
        st_pool = ctx.enter_context(tc.tile_pool(name="st", bufs=3))
        psum_pool = ctx.enter_context(
            tc.tile_pool(name="psum", bufs=2, space="PSUM"))

        # --- constants ---
        # iota over b bins, b-outer/row-inner: value k at [p, k*G + r]
        iota_b_i = const_pool.tile([P, NB * G], i16)
        nc.gpsimd.iota(iota_b_i[:], pattern=[[1, NB], [0, G]],
                       channel_multiplier=0)
        iota_b = const_pool.tile([P, NB * G], bf16)
        nc.vector.tensor_copy(iota_b[:], iota_b_i[:])
        # row offsets for a-side scatter indices: 32*(r % 16) over 128 cols
        rowoff = const_pool.tile([P, RPB], i16)
        nc.gpsimd.iota(rowoff[:], pattern=[[0, RPB // G], [NA, G]],
                       channel_multiplier=0)
        ones_t = const_pool.tile([P, G], bf16)
        nc.vector.memset(ones_t[:], 1.0)

        counts_v = counts.rearrange("(n f) t -> n f t", f=4)  # [512, 4, 1100]

        for bi in range(n_batches):
            # --- load + derive a, b ---
            xt = x_pool.tile([P, S], i32)
            nc.sync.dma_start(out=xt[:], in_=x[bi * RPB:(bi + 1) * RPB, :])
            # a = x // 35 via magic-number division: (x * 937) >> 15,
            # exact for all x < 42477; b = x - 35 * a.
            xm = deriv_pool.tile([P, S], i32)
            nc.vector.tensor_scalar(xm[:], xt[:], 937, None,
                                    op0=mybir.AluOpType.mult)
            a_i = deriv_pool.tile([P, S], i32)
            nc.vector.tensor_scalar(a_i[:], xm[:], 15, None,
                                    op0=mybir.AluOpType.logical_shift_right)
            a_bf = deriv_pool.tile([P, S], bf16)
            nc.vector.tensor_copy(a_bf[:], a_i[:])
            b_bf = deriv_pool.tile([P, S], bf16)
            nc.vector.scalar_tensor_tensor(
                b_bf[:], a_i[:], -35.0, xt[:],
                op0=mybir.AluOpType.mult, op1=mybir.AluOpType.add)

            # --- transpose a, b to [s, row] ---
            aT = []
            bT = []
            for c in range(4):
                at = tr_pool.tile([P, RPB], bf16, tag="aT")
                nc.sync.dma_start(out=at[:],
                                  in_=a_bf[:, c * P:(c + 1) * P],
                                  transpose=True)
                aT.append(at)
                bt = tr_pool.tile([P, RPB], bf16, tag="bT")
                nc.sync.dma_start(out=bt[:],
                                  in_=b_bf[:, c * P:(c + 1) * P],
                                  transpose=True)
                bT.append(bt)

            # a-side scatter indices for all rows of each chunk: 32*(r%16)+a
            idx = []
            for c in range(4):
                ix = tr_pool.tile([P, RPB], i16, tag="idx")
                nc.vector.tensor_tensor(ix[:], aT[c][:], rowoff[:],
                                        op=mybir.AluOpType.add)
                idx.append(ix)

            st = st_pool.tile([P, 140 * 32], fp16)

            for g in range(RPB // G):
                # --- one-hots for this 16-row group, all 4 s-chunks ---
                oa = []
                ob = []
                for c in range(4):
                    o_a = oh_pool.tile([P, G * NA], bf16, tag="oa")
                    nc.gpsimd.local_scatter(
                        out_ap=o_a[:],
                        data_ap=ones_t[:],
                        idxs_ap=idx[c][:, g * G:(g + 1) * G],
                        channels=P, num_elems=G * NA, num_idxs=G)
                    oa.append(o_a)
                    o_b = oh_pool.tile([P, NB * G], bf16, tag="ob")
                    bsl = bT[c][:, g * G:(g + 1) * G]
                    nc.vector.tensor_tensor(
                        o_b[:], bsl[:, None, :].to_broadcast([P, NB, G]),
                        iota_b[:], op=mybir.AluOpType.is_equal)
                    ob.append(o_b)
                for wave in range(G // 16):
                    pss = []
                    for m in range(4):
                        ps_t = psum_pool.tile([P, 140], f32, space="PSUM",
                                              tag=f"ps{m}", name=f"ps{m}")
                        pss.append(ps_t)
                    for c in range(4):
                        for m in range(4):
                            mg = wave * 4 + m
                            # moving columns streamed in native (k, r) order:
                            # innermost dim contiguous in SBUF (fast fetch);
                            # the PSUM->SBUF copy undoes the permutation.
                            rhs = ob[c][:].rearrange(
                                "p (k r) -> p k r",
                                k=NB)[:, :, mg * 4:(mg + 1) * 4]
                            nc.tensor.matmul(
                                pss[m][:],
                                lhsT=oa[c][:, mg * P:(mg + 1) * P],
                                rhs=rhs,
                                start=(c == 0), stop=(c == 3))
                    for m in range(4):
                        grp = g * (G // 4) + wave * 4 + m
                        st_sl = st[:, 140 * grp:140 * (grp + 1)].rearrange(
                            "p (r k) -> p r k", r=4)
                        nc.scalar.copy(
                            st_sl,
                            pss[m][:].rearrange("p (k r) -> p r k", k=NB))

            # --- scatter diagonal blocks to HBM ---
            stv = st[:].rearrange("p (grp r k) -> p grp r k", grp=32, r=4)
            cb = counts_v[bi * 32:(bi + 1) * 32]  # [32, 4, 1100]
            for j in range(4):
                # a in [1, 31): 30 partitions x 35 cols -> t-1 in [34, 1084)
                dst = cb[:, j, 34:1084].rearrange("r (a b) -> a r b", a=30)
                nc.sync.dma_start(
                    out=dst, in_=stv[32 * j + 1:32 * j + 31, :, j, :])
                # a == 0: b in [1, 35) -> t-1 in [0, 34)
                nc.sync.dma_start(
                    out=cb[None, :, j, 0:34],
                    in_=stv[32 * j:32 * j + 1, :, j, 1:35])
                # a == 31: b in [0, 16) -> t-1 in [1084, 1100)
                nc.sync.dma_start(
                    out=cb[None, :, j, 1084:1100],
                    in_=stv[32 * j + 31:32 * j + 32, :, j, 0:16])


_NC_CACHE = {}


def _get_nc():
    if "nc" not in _NC_CACHE:
        _NC_CACHE["nc"] = build_nc()
    return _NC_CACHE["nc"]


def kernel(**inputs):
    x = np.asarray(inputs["inputs"])
    in_dtype = x.dtype
    x = np.ascontiguousarray(x.astype(np.int32))
    shards = x.reshape(N_CORES, B_CORE, S)
    nc = _get_nc()
    in_maps = [{"x": shards[i]} for i in range(N_CORES)]
    res = run_bass_kernel_spmd(nc, in_maps, core_ids=list(range(N_CORES)))
    out = np.concatenate([r["counts"] for r in res.results], axis=0)
    return out.astype(np.float32)


if __name__ == "__main__":
    rng = np.random.default_rng(0)
    x = rng.integers(0, 1101, size=(16384, 512), dtype=np.int32)
    out = kernel(inputs=x)
    # numpy reference
    exp = np.zeros((16384, 1101), np.float32)
    for r in range(0, 16384, 4096):
        blk = x[r:r + 4096]
        idx = np.arange(blk.shape[0])[:, None]
        np.add.at(exp[r:r + 4096], (idx, blk), 1.0)
    exp = exp[:, 1:]
    print("match:", np.array_equal(out, exp),
          "maxerr:", np.abs(out - exp).max())
